# revision 50
# baseline (speedup 1.0000x reference)
"""Trainium2 Bass kernel for nn_MultiHeadAttentionQuantum.

Math: the per-(batch,token,head) quantum circuit (RX(x_i+theta_i) encode, CNOT
ring, <Z_i> readout) collapses analytically via Heisenberg/Clifford conjugation:
    <Z_0> = prod_{i=1..7} cos(x_i + theta_i)
    <Z_w> = prod_{i=0..w} cos(x_i + theta_i)   (w >= 1)
so the "quantum head" is cosine prefix-products. Downstream it is a plain
16-head self-attention (q=k=v, d_k=8, no max-subtraction needed since
|score| <= sqrt(8)) plus an output projection.

Sharding: data-parallel over batch, one batch element per NeuronCore (B=8,
n_cores=8). combine_heads weights replicated. No collectives.
"""

import math
import sys

sys.path.insert(0, "/opt/trn_rl_repo")

import numpy as np

import concourse.bass as bass  # noqa: F401  (import keeps bass registered)
import concourse.tile as tile
from concourse import bacc, mybir
from concourse import bass_utils

FP32 = mybir.dt.float32
FP16 = mybir.dt.float16
AF = mybir.ActivationFunctionType


def _r(ap):
    return ap

B, S, E, H, NW = 8, 512, 128, 16, 8   # batch, seq, embed, heads, wires(d_k)
TB = S // 128                         # token blocks per core = 4
ISQ = 1.0 / math.sqrt(NW)             # 1/sqrt(d_k) folded into the exp scale

_CACHE = {}


def build(repeat: int = 1):
    """Build + compile the per-core Bass program. Cached per `repeat`."""
    if repeat in _CACHE:
        return _CACHE[repeat]

    nc = bacc.Bacc("TRN2", target_bir_lowering=False, debug=False, num_devices=8)

    xin_d = nc.dram_tensor("xin", [128, 512], FP32, kind="ExternalInput").ap()
    idn_d = nc.dram_tensor("idn", [128, 128], FP32, kind="ExternalInput").ap()
    selz_d = nc.dram_tensor("selz", [128, 128], FP16, kind="ExternalInput").ap()
    msk_d = nc.dram_tensor("msk", [128, 4], FP32, kind="ExternalInput").ap()
    wtb_d = nc.dram_tensor("wtb", [128, 512], FP16, kind="ExternalInput").ap()
    bvec_d = nc.dram_tensor("bvec", [1, 128], FP16, kind="ExternalInput").ap()
    zc_d = nc.dram_tensor("zc", [1, 512], FP16, kind="ExternalInput").ap()
    ones5_d = nc.dram_tensor("ones5", [1, 512], FP16, kind="ExternalInput").ap()
    yout_d = nc.dram_tensor("yout", [128, 512], FP32, kind="ExternalOutput").ap()

    with tile.TileContext(nc) as tc:
        with tc.tile_pool(name="consts", bufs=1) as cpool, \
             tc.tile_pool(name="sb", bufs=1) as spool, \
             tc.tile_pool(name="Pp", bufs=4) as Ppool, \
             tc.tile_pool(name="uTp", bufs=2) as uTpool, \
             tc.tile_pool(name="rzp", bufs=2) as rzpool, \
             tc.tile_pool(name="yop", bufs=2) as yopool, \
             tc.tile_pool(name="psS", bufs=2, space="PSUM") as psS, \
             tc.tile_pool(name="psU", bufs=2, space="PSUM") as psU, \
             tc.tile_pool(name="psZ", bufs=1, space="PSUM") as psZ:

            for rep in range(repeat):
                # ---- input first (sync + gpsimd queues), then consts on gpsimd
                X = spool.tile([128, 512], FP32, tag="X")
                nc.sync.dma_start(X[:, 0:256], xin_d[:, 0:256])
                nc.gpsimd.dma_start(X[:, 256:512], xin_d[:, 256:512])
                idn = cpool.tile([128, 128], FP32, tag="idn")
                nc.gpsimd.dma_start(idn[:], idn_d[:])
                zc = cpool.tile([1, 512], FP16, tag="zc")
                nc.gpsimd.dma_start(zc[:], zc_d[:])
                selz = cpool.tile([128, 128], FP16, tag="selz")
                nc.sync.dma_start(selz[:], selz_d[:])
                msk = cpool.tile([128, 4], FP32, tag="msk")
                nc.sync.dma_start(msk[:], msk_d[:])
                wtb = cpool.tile([128, 512], FP16, tag="wtb")
                nc.sync.dma_start(wtb[:], wtb_d[:])
                bvec = cpool.tile([1, 128], FP16, tag="bvec")
                nc.sync.dma_start(bvec[:], bvec_d[:])
                ones5 = cpool.tile([1, 512], FP16, tag="ones5")
                nc.sync.dma_start(ones5[:], ones5_d[:])

                # ---- xin already holds C = cos(x + theta) (host-encoded angles)
                C = X

                # prefix products -> XQ (natural [p, (tb, h, w)] layout, stride 8)
                XQ = spool.tile([128, 512], FP32, tag="XQ")
                Cr = C[:].rearrange("p (t h w) -> p t h w", t=TB, h=H, w=NW)
                Qr = XQ[:].rearrange("p (t h w) -> p t h w", t=TB, h=H, w=NW)
                # cum chain (Hillis-Steele, log depth): XQ[w] = prod_{0..w} C
                nc.vector.tensor_copy(Qr[:, :, :, :], Cr[:, :, :, :])
                for st in (1, 2, 4):
                    nc.vector.tensor_mul(
                        Qr[:, :, :, st:NW], Qr[:, :, :, st:NW], Qr[:, :, :, 0:NW - st]
                    )
                # wire 0 = suffix prod_{1..7} C, via 3-level tree in scratch cols
                # t_a = c1*c2 -> XQ0 ; t_b = c3*c4, t_c = c5*c6 in scratch tile
                scrT = spool.tile([128, 256], FP32, tag="scrT")
                Tr = scrT[:].rearrange("p (t h w) -> p t h w", t=TB, h=H, w=4)
                nc.gpsimd.tensor_mul(Tr[:, :, :, 0:1], Cr[:, :, :, 3:4], Cr[:, :, :, 4:5])
                nc.gpsimd.tensor_mul(Tr[:, :, :, 1:2], Cr[:, :, :, 5:6], Cr[:, :, :, 6:7])
                nc.gpsimd.tensor_mul(Tr[:, :, :, 2:3], Cr[:, :, :, 1:2], Cr[:, :, :, 2:3])
                nc.gpsimd.tensor_mul(Tr[:, :, :, 3:4], Tr[:, :, :, 0:1], Tr[:, :, :, 1:2])
                nc.gpsimd.tensor_mul(Tr[:, :, :, 3:4], Tr[:, :, :, 3:4], Cr[:, :, :, 7:8])
                nc.gpsimd.tensor_mul(Qr[:, :, :, 0:1], Tr[:, :, :, 2:3], Tr[:, :, :, 3:4])

                # transposes -> xqT [e, (tb, s)]
                xqT = spool.tile([128, 512], FP16, tag="xqT")
                pst = psZ.tile([128, 512], FP32, tag="psZ_g", name="pst")
                for tb in range(TB):
                    nc.tensor.transpose(
                        pst[:, 128 * tb:128 * (tb + 1)],
                        XQ[:, 128 * tb:128 * (tb + 1)], idn[:],
                    )
                nc.vector.tensor_copy(xqT[:], pst[:])

                # masked variants for per-head K=32 score matmuls; v=0 heads are
                # 32-aligned in xqT and use direct K=8 slabs (no mask needed)
                Mv = [None]
                for v in range(1, 4):
                    m = spool.tile([128, 512], FP16, tag=f"Mv{v}", name=f"Mv{v}")
                    nc.vector.tensor_scalar_mul(m[:], xqT[:], msk[:, v:v + 1])
                    Mv.append(m)

                # VP: [p, (tb, h, w0..7, one)] stride-9 layout for PV lhsT slabs
                VP = spool.tile([128, 576], FP16, tag="VP")
                VPr = VP[:].rearrange("p (t h w) -> p t h w", t=TB, h=H, w=NW + 1)
                nc.vector.tensor_copy(VPr[:, :, :, 0:NW], Qr[:, :, :, :])
                nc.vector.tensor_scalar(
                    VPr[:, :, :, NW:NW + 1], Cr[:, :, :, 0:1], 0.0, 1.0,
                    mybir.AluOpType.mult, mybir.AluOpType.add,
                )

                # ---- attention
                xoT = [spool.tile([128, 512], FP16, tag=f"xoT{g}", name=f"xoT{g}") for g in range(4)]
                psOT = psU.tile([128, 512], FP32, tag="psOTa", name="psOT", bufs=1)
                psU_g = None
                psU_prev = None

                def emit_pv(hh, psU_ref):
                    gg, vv = hh // 4, hh % 4
                    for c in range(TB):
                        nc.tensor.matmul(
                            psU_ref[32 * vv:32 * vv + 9, :],
                            _r(VP[:, 144 * c + 9 * hh:144 * c + 9 * hh + 9]),
                            _r(Phs[hh][:, 512 * c:512 * (c + 1)]),
                            start=False, stop=(vv == 3 and c == TB - 1),
                            tile_position=(0, 32 * vv), skip_group_check=True,
                        )

                def emit_setchain(gg):
                    uT = uTpool.tile([128, 512], FP16, tag="uT", name=f"uT{gg}")
                    nc.vector.tensor_copy(uT[:], psUs[gg][:])
                    psZ_g = psZ.tile([128, 512], FP32, tag="psZ_g", name=f"psZg{gg}")
                    nc.tensor.matmul(psZ_g[:], _r(selz[:]), _r(uT[:]), start=True, stop=True)
                    rz = rzpool.tile([128, 512], FP32, tag="rz", name=f"rz{gg}")
                    nc.vector.reciprocal_approx_fast(out=rz[:], in_=psZ_g[:])
                    nc.vector.tensor_mul(xoT[gg][:], uT[:], rz[:])
                    nc.tensor.matmul(
                        psOT[:], wtb[:, 128 * gg:128 * (gg + 1)], xoT[gg][:],
                        start=(gg == 0), stop=False, skip_group_check=True,
                    )
                    if gg == 0:
                        nc.tensor.matmul(
                            psOT[:], bvec[:1, :], ones5[:1, :],
                            start=False, stop=False, skip_group_check=True,
                        )

                Phs = {}
                psUs = {}
                for h in range(H):
                    g, v = h // 4, h % 4
                    Ph = Ppool.tile([128, 2048], FP16, tag="Ph", name=f"Ph{h}")
                    Phs[h] = Ph
                    for half in range(2):
                        ps_s = psS.tile([128, 1024], FP32, tag="ps_s")
                        for j in range(2):
                            a = 2 * half + j
                            if v == 0:
                                lhsT = xqT[32 * g:32 * g + 8, 128 * a:128 * (a + 1)]
                                rhs = xqT[32 * g:32 * g + 8, :]
                            else:
                                lhsT = Mv[v][32 * g:32 * (g + 1), 128 * a:128 * (a + 1)]
                                rhs = xqT[32 * g:32 * (g + 1), :]
                            nc.tensor.matmul(
                                ps_s[:, 512 * j:512 * (j + 1)], lhsT, rhs,
                                start=True, stop=True,
                                tile_position=(32 * g, 0),
                            )
                        nc.scalar.activation(
                            Ph[:, 1024 * half:1024 * (half + 1)], ps_s[:], AF.Exp,
                            scale=ISQ,
                        )
                    if v == 0:
                        psU_g = psU.tile([128, 512], FP32, tag="psU_g", name=f"psU{g}")
                        psUs[g] = psU_g
                        nc.tensor.matmul(
                            psU_g[:], _r(zc[:1, 0:128]), _r(zc[:1, 0:512]),
                            start=True, stop=False, skip_group_check=True,
                        )
                    # PV for the PREVIOUS head (one-head delay keeps scores feeding ACT)
                    if h > 0:
                        emit_pv(h - 1, psUs[(h - 1) // 4])
                        if (h - 1) % 4 == 3:
                            emit_setchain((h - 1) // 4)
                    if h == H - 1:
                        emit_pv(h, psUs[3])
                        emit_setchain(3)

                # ---- writeback (bias folded in after set 0)
                yo = yopool.tile([128, 512], FP32, tag="yo")
                nc.vector.tensor_copy(yo[:, 0:256], psOT[:, 0:256])
                nc.sync.dma_start(yout_d[:, 0:256], yo[:, 0:256])
                nc.vector.tensor_copy(yo[:, 256:512], psOT[:, 256:512])
                nc.gpsimd.dma_start(yout_d[:, 256:512], yo[:, 256:512])

    nc.compile()
    _CACHE[repeat] = nc
    return nc


def _consts(W: np.ndarray, b: np.ndarray):
    idn = np.eye(128, dtype=np.float32)
    selz = np.zeros((128, 128), dtype=np.float32)
    for m in range(128):
        selz[32 * (m // 32) + 8, m] = 1.0
    msk = np.zeros((128, 4), dtype=np.float32)
    for p in range(128):
        msk[p, (p % 32) // 8] = 1.0
    # wtb[32t+d, 128s+e'] = W[e', 8*(4s+t)+d]  (d<8); Z rows / pad rows zero
    wtb = np.zeros((128, 512), dtype=np.float32)
    for s in range(4):
        for t in range(4):
            head = 4 * s + t
            wtb[32 * t:32 * t + 8, 128 * s:128 * (s + 1)] = W[:, 8 * head:8 * head + 8].T
    bvec = b.reshape(1, 128).astype(np.float16)
    ones5 = np.ones((1, 512), dtype=np.float16)
    zc = np.zeros((1, 512), dtype=np.float16)
    return {
        "idn": idn, "selz": selz.astype(np.float16), "msk": msk,
        "wtb": wtb.astype(np.float16),
        "bvec": bvec, "ones5": ones5, "zc": zc,
    }


def _prep_x(x: np.ndarray, theta: np.ndarray) -> list[np.ndarray]:
    """Per-core xin: RX-encoding cosines cos(x + theta), laid out as
    [token_within_block, (block, embed)]."""
    theta_full = np.tile(theta.astype(np.float64), E // NW)
    a = np.cos(x.astype(np.float64) + theta_full).astype(np.float32)
    return [
        np.ascontiguousarray(
            a[bb].reshape(TB, 128, E).transpose(1, 0, 2).reshape(128, TB * E)
        )
        for bb in range(B)
    ]


def kernel(x: np.ndarray, theta: np.ndarray, W: np.ndarray, b: np.ndarray) -> np.ndarray:
    x = np.asarray(x, dtype=np.float32)
    theta = np.asarray(theta, dtype=np.float32)
    W = np.asarray(W, dtype=np.float32)
    b = np.asarray(b, dtype=np.float32)

    nc = build(repeat=1)
    consts = _consts(W, b)
    xins = _prep_x(x, theta)
    in_maps = [{**consts, "xin": xins[c]} for c in range(B)]
    res = bass_utils.run_bass_kernel_spmd(nc, in_maps, core_ids=list(range(8)))

    y = np.empty((B, S, E), dtype=np.float32)
    for c in range(B):
        y[c] = res.results[c]["yout"].T  # [e', q] -> [q, e']
    return y
